# revision 8
# baseline (speedup 1.0000x reference)
"""Trainium2 Bass kernel for ActivationRealQuantLinear.

Math (reference):
  per-token asymmetric 8-bit activation quant:
    xs = clip((max-min)/255, 1e-5), zp = clip(round(-min/xs), 0, 255)
    cx = clip(round(x/xs), -zp, 255-zp)            (integers in [-255,255])
  grouped uint4 weight dequant: wdq[o,k] = (qw[o,k] - wzp[o,g]) * wsc[o,g]
  out[s,o] = (cx @ wdq.T)[s,o] * xs[s] + bias[o]

Distribution (8 NeuronCores, one TRN2 chip):
  - out_features tensor-parallel: each core owns a 512-wide o-slice of
    qweight/scales/zero_points/bias and computes out[:, o_slice].
  - activation quant is token-sharded: each core quantizes 256 tokens,
    transposes to [k, s] layout (one xbar DMA-transpose per 128-token
    half), and AllGathers the quantized activations (bf16, exact
    integers). The per-token scale rides along in the gathered buffer as
    a bf16 hi+lo pair (slot KC), so no second collective is needed.
  - the gather is split into two collectives (one per 128-token half) so
    matmul on half 0 overlaps the gather of half 1.
  - matmul in bf16 (cx exact in bf16; dequantized weights rounded to
    bf16, ~2^-9 relative error), fp32 PSUM accumulation.
"""

import os
import sys

if "/opt/trn_rl_repo" not in sys.path:
    sys.path.insert(0, "/opt/trn_rl_repo")

import numpy as np
import ml_dtypes

import concourse.bacc as bacc
import concourse.mybir as mybir
import concourse.tile as tile
from concourse.bass_utils import run_bass_kernel_spmd

NCORES = 8
S, K, O = 2048, 4096, 4096
SL = S // NCORES          # 256 tokens quantized per core
OL = O // NCORES          # 512 out features per core
G = 32                    # weight quant groups
KC = K // 128             # 32 k-chunks of 128
MAGIC = float(1.5 * 2 ** 23)   # fp32 round-to-nearest-even trick
F32 = mybir.dt.float32
BF16 = mybir.dt.bfloat16

_GRAPH = None
LAST_RESULTS = None


def _build():
    nc = bacc.Bacc("TRN2", target_bir_lowering=False, debug=False,
                   num_devices=NCORES)

    x_p = nc.declare_dram_parameter("x_loc", [SL, K], F32, isOutput=False)
    qw_p = nc.declare_dram_parameter("qw", [OL, K], BF16, isOutput=False)
    wsc_p = nc.declare_dram_parameter("wsc", [OL, G], F32, isOutput=False)
    wzp_p = nc.declare_dram_parameter("wzp", [OL, G], F32, isOutput=False)
    b_p = nc.declare_dram_parameter("bias", [1, OL], F32, isOutput=False)
    out_p = nc.declare_dram_parameter("out", [S, OL], F32, isOutput=True)

    # per-half gather buffers; slot KC of the chunk dim carries xs hi/lo
    cxt_loc = [nc.dram_tensor(f"cxt_loc{h}", [128, KC + 1, 128], BF16)
               for h in range(2)]
    cxt_all = [nc.dram_tensor(f"cxt_all{h}", [NCORES, 128, KC + 1, 128],
                              BF16, addr_space="Shared") for h in range(2)]

    groups = [list(range(NCORES))]
    Alu = mybir.AluOpType

    with tile.TileContext(nc) as tc:
        with (
            tc.tile_pool(name="persist", bufs=1) as persist,
            tc.tile_pool(name="xin", bufs=2) as xinp,
            tc.tile_pool(name="xtile", bufs=1) as xpool,
            tc.tile_pool(name="wtile", bufs=2) as wpool,
            tc.tile_pool(name="small", bufs=4) as small,
            tc.tile_pool(name="mm", bufs=3) as mmp,
            tc.tile_pool(name="out", bufs=3) as opool,
            tc.tile_pool(name="psum", bufs=4, space="PSUM") as psp,
        ):
            # ------- persistent tiles -------
            wdqT = persist.tile([128, KC, OL], BF16)        # 4 MB resident
            ones_col = persist.tile([1, 128], F32)
            nc.vector.memset(ones_col[:], 1.0)
            bias_bcast = persist.tile([128, OL], F32)

            # ------- phase 1: quantize own 256 tokens, per 128-token half --
            for h in range(2):
                x_t = xinp.tile([128, K], F32, tag="xf32")
                nc.sync.dma_start(out=x_t[:], in_=x_p[h * 128:(h + 1) * 128, :])

                xmin = small.tile([128, 1], F32, tag="st")
                xmax = small.tile([128, 1], F32, tag="st")
                nc.vector.tensor_reduce(xmin[:], x_t[:], mybir.AxisListType.X,
                                        Alu.min)
                nc.vector.tensor_reduce(xmax[:], x_t[:], mybir.AxisListType.X,
                                        Alu.max)
                xs = small.tile([128, 1], F32, tag="st")
                nc.vector.tensor_sub(xs[:], xmax[:], xmin[:])
                nc.vector.tensor_scalar(xs[:], xs[:], 1.0 / 255.0, 1e-5,
                                        Alu.mult, Alu.max)
                # reciprocal + one Newton step
                r = small.tile([128, 1], F32, tag="st")
                nc.vector.reciprocal(r[:], xs[:])
                t = small.tile([128, 1], F32, tag="st")
                nc.vector.tensor_mul(t[:], xs[:], r[:])
                nc.vector.tensor_scalar(t[:], t[:], 2.0, -1.0,
                                        Alu.subtract, Alu.mult)  # 2 - xs*r
                nc.vector.tensor_mul(r[:], r[:], t[:])
                # zp = clip(round(-xmin*r), 0, 255); lo = -zp; hi = 255-zp
                zp = small.tile([128, 1], F32, tag="st")
                nc.vector.tensor_scalar(zp[:], xmin[:], -1.0, None, Alu.mult)
                nc.vector.tensor_mul(zp[:], zp[:], r[:])
                nc.vector.tensor_scalar(zp[:], zp[:], MAGIC, MAGIC,
                                        Alu.add, Alu.subtract)
                nc.vector.tensor_scalar(zp[:], zp[:], 0.0, 255.0,
                                        Alu.max, Alu.min)
                lo = small.tile([128, 1], F32, tag="st")
                hi = small.tile([128, 1], F32, tag="st")
                nc.vector.tensor_scalar(lo[:], zp[:], -1.0, None, Alu.mult)
                nc.vector.tensor_scalar(hi[:], zp[:], -1.0, 255.0,
                                        Alu.mult, Alu.add)
                # cx = clip(round(x*r), lo, hi)  (round via magic constant)
                tq = xpool.tile([128, K], F32, tag="tq")
                nc.vector.tensor_scalar(tq[:], x_t[:], r[:], MAGIC,
                                        Alu.mult, Alu.add)
                nc.vector.tensor_scalar(tq[:], tq[:], MAGIC, lo[:],
                                        Alu.subtract, Alu.max)
                cx_sb = xpool.tile([128, K], BF16, tag="cx")
                nc.vector.tensor_scalar(cx_sb[:], tq[:], hi[:], None, Alu.min)

                # one xbar transpose: [128 s, 4096 k] -> [128 kp, KC, 128 s]
                cxT = xpool.tile([128, KC + 1, 128], BF16, tag="cxT")
                nc.sync.dma_start(out=cxT[:, 0:KC, :], in_=cx_sb[:],
                                  transpose=True)
                # xs as bf16 hi/lo pair in slot KC, columns 0 and 1
                xs_hi_bf = small.tile([128, 1], BF16, tag="sb")
                xs_hi_f = small.tile([128, 1], F32, tag="st")
                nc.vector.tensor_copy(xs_hi_bf[:], xs[:])
                nc.vector.tensor_copy(xs_hi_f[:], xs_hi_bf[:])
                nc.vector.tensor_copy(cxT[:, KC, 0:1], xs_hi_bf[:])
                nc.vector.tensor_sub(cxT[:, KC, 1:2], xs[:], xs_hi_f[:])

                nc.sync.dma_start(out=cxt_loc[h][:], in_=cxT[:])
                nc.gpsimd.collective_compute(
                    "AllGather", Alu.bypass, replica_groups=groups,
                    ins=[cxt_loc[h][:]], outs=[cxt_all[h][:]])

            # ------- phase 2: dequantize own weight slice (overlaps gather) --
            for oc in range(4):
                qw_t = wpool.tile([128, K], BF16, tag="qw")
                nc.sync.dma_start(out=qw_t[:],
                                  in_=qw_p[oc * 128:(oc + 1) * 128, :])
                wsc_t = small.tile([128, G], F32, tag="wsb")
                wzp_t = small.tile([128, G], F32, tag="wsb")
                nc.sync.dma_start(out=wsc_t[:],
                                  in_=wsc_p[oc * 128:(oc + 1) * 128, :])
                nc.sync.dma_start(out=wzp_t[:],
                                  in_=wzp_p[oc * 128:(oc + 1) * 128, :])
                nps = small.tile([128, G], F32, tag="wsb")
                nc.vector.tensor_mul(nps[:], wzp_t[:], wsc_t[:])
                nc.vector.tensor_scalar(nps[:], nps[:], -1.0, None, Alu.mult)
                wdq = wpool.tile([128, K], BF16, tag="wdq")
                for g in range(G):
                    sl = slice(g * 128, (g + 1) * 128)
                    if g % 2 == 0:
                        nc.vector.tensor_scalar(
                            wdq[:, sl], qw_t[:, sl], wsc_t[:, g:g + 1],
                            nps[:, g:g + 1], Alu.mult, Alu.add)
                    else:
                        nc.scalar.activation(
                            wdq[:, sl], qw_t[:, sl],
                            mybir.ActivationFunctionType.Identity,
                            bias=nps[:, g:g + 1], scale=wsc_t[:, g:g + 1])
                # one xbar transpose into the resident [k, o] weight tile
                nc.sync.dma_start(
                    out=wdqT[:, :, oc * 128:(oc + 1) * 128],
                    in_=wdq[:], transpose=True)

            # ------- phase 3: bias broadcast (PE outer product) -------
            b_row = small.tile([1, OL], F32, tag="brow")
            nc.sync.dma_start(out=b_row[:], in_=b_p[:])
            ps_b = psp.tile([128, OL], F32, tag="ps")
            nc.tensor.matmul(ps_b[:], ones_col[:], b_row[:],
                             start=True, stop=True)
            nc.vector.tensor_copy(bias_bcast[:], ps_b[:])

            # ------- phase 4: matmul over all 2048 tokens, half 0 first ----
            for hh in range(2):
                for c in range(NCORES):
                    ti = 2 * c + hh
                    lhsT = mmp.tile([128, KC + 1, 128], BF16, tag="lhsT")
                    nc.sync.dma_start(out=lhsT[:], in_=cxt_all[hh][c])
                    ps = psp.tile([128, OL], F32, tag="ps")
                    for kc in range(KC):
                        nc.tensor.matmul(ps[:], lhsT[:, kc, :],
                                         wdqT[:, kc, :],
                                         start=(kc == 0),
                                         stop=(kc == KC - 1))
                    xs_t = small.tile([128, 1], F32, tag="xst")
                    nc.vector.tensor_add(xs_t[:], lhsT[:, KC, 0:1],
                                         lhsT[:, KC, 1:2])
                    o_t = opool.tile([128, OL], F32, tag="ot")
                    nc.vector.tensor_scalar(o_t[:], ps[:], xs_t[:],
                                            None, Alu.mult)
                    nc.vector.tensor_add(o_t[:], o_t[:], bias_bcast[:])
                    nc.sync.dma_start(out=out_p[ti * 128:(ti + 1) * 128, :],
                                      in_=o_t[:])

    nc.compile()
    return nc


def _get_graph():
    global _GRAPH
    if _GRAPH is None:
        _GRAPH = _build()
    return _GRAPH


def kernel(x, qweight, w_scales, w_zero_points, bias):
    global LAST_RESULTS
    x2 = np.ascontiguousarray(np.asarray(x, np.float32).reshape(S, K))
    qw = np.ascontiguousarray(
        np.asarray(qweight).astype(ml_dtypes.bfloat16).reshape(O, K))
    wsc = np.ascontiguousarray(np.asarray(w_scales, np.float32))
    wzp = np.ascontiguousarray(np.asarray(w_zero_points).astype(np.float32))
    b = np.ascontiguousarray(np.asarray(bias, np.float32).reshape(1, O))

    in_maps = []
    for c in range(NCORES):
        in_maps.append({
            "x_loc": np.ascontiguousarray(x2[c * SL:(c + 1) * SL]),
            "qw": np.ascontiguousarray(qw[c * OL:(c + 1) * OL]),
            "wsc": np.ascontiguousarray(wsc[c * OL:(c + 1) * OL]),
            "wzp": np.ascontiguousarray(wzp[c * OL:(c + 1) * OL]),
            "bias": np.ascontiguousarray(b[:, c * OL:(c + 1) * OL]),
        })

    nc = _get_graph()
    trace = os.environ.get("KTRACE", "0") == "1"
    res = run_bass_kernel_spmd(nc, in_maps, core_ids=list(range(NCORES)),
                               trace=trace)
    LAST_RESULTS = res
    outs = [np.asarray(res.results[c]["out"]) for c in range(NCORES)]
    return np.concatenate(outs, axis=1).reshape(1, S, O).astype(np.float32)


if __name__ == "__main__":
    rng = np.random.default_rng(0)
    x = rng.standard_normal((1, S, K), dtype=np.float32)
    qweight = rng.integers(0, 16, (O, G, 128), dtype=np.int32)
    w_scales = rng.uniform(0.001, 0.02, (O, G)).astype(np.float32)
    w_zero_points = rng.integers(0, 16, (O, G), dtype=np.int32)
    bias = rng.standard_normal(O).astype(np.float32)
    out = kernel(x=x, qweight=qweight, w_scales=w_scales,
                 w_zero_points=w_zero_points, bias=bias)
    print("out", out.shape, out.dtype, out[0, :2, :4])
